# revision 1
# baseline (speedup 1.0000x reference)
"""Trainium2 Bass kernel for nn_BurgersSolver_75333726371954.

Burgers' equation explicit solver: interpolate u0 [64,512] to a 513-point
grid, run 5000 sequential periodic-stencil steps on [64,512], snapshot every
50th step at every 2nd spatial point -> [64,257,101].

Strategy (pure data parallel, batch sharded 8 rows/core across 8 cores):
  * Scaled state w = C1*u so the update is
        w' = (w+C2)*w_left - (w-C2)*w_right + (1-2*C2)*w
    = 4 standard DVE ops/step (2x scalar_tensor_tensor, tensor_sub, STT).
  * Layout [128 partitions = 8 batch x 16 spatial chunks of 32,
    free = 32 + 2H ghost columns]. Ghost zones allow H steps between
    partition-crossing halo exchanges; compute range tapers by 1/side/step.
  * Halo exchange via two TensorE permutation matmuls (bit-exact for fp32)
    into PSUM + one strided PSUM->SBUF copy, every H steps.
  * Snapshots: strided 1x-mode DVE tensor-add (copy) of the 16 even-spatial
    valid columns into an SBUF accumulation area; single DMA out at the end;
    host rescales by 1/C1 and assembles the [64,257,101] output.
  * A one-op writeback-margin spacer follows every in-place state update
    (DVE streaming reads at equal rate catch the previous op's writeback).
"""

import numpy as np

# ---- problem constants (hardcoded; must match the reference config) ----
MX = 513
MT = 5001
DX = 1.0 / (MX - 1)
DT = 1.0 / (MT - 1)
C1 = DT / (2.0 * DX)            # 0.0512
C2 = 0.005 * DT / DX ** 2       # 0.262144
LIN = float(1.0 - 2.0 * C2)

NSTEPS = MT - 1                 # 5000
SNAP_EVERY = 50
NSNAP = NSTEPS // SNAP_EVERY + 1  # 101

NCORES = 8
BPC = 8                         # batch rows per core
NCHUNK = 16                     # spatial chunks per batch row
CH = 32                         # chunk width (NCHUNK*CH == 512)
H = 20                         # ghost depth == steps between exchanges
W = CH + 2 * H                  # tile free width

_COMPILED = {}


def _build():
    import concourse.bass as bass
    import concourse.mybir as mybir

    F32 = mybir.dt.float32
    ALU = mybir.AluOpType

    nc = bass.Bass()
    x_in = nc.dram_tensor("x", [128, W], F32, kind="ExternalInput")
    pm_in = nc.dram_tensor("pm", [128, 256], F32, kind="ExternalInput")
    y_out = nc.dram_tensor("y", [128, NSNAP * 16], F32, kind="ExternalOutput")

    n_blocks = (NSTEPS + H - 1) // H
    assert NSTEPS % H == 0

    with (
        nc.semaphore("dma_sem") as dma_sem,
        nc.semaphore("x_sem") as x_sem,
        nc.semaphore("p_sem") as p_sem,
        nc.semaphore("v_sem") as v_sem,
        nc.sbuf_tensor("U", [128, W], F32) as U,
        nc.sbuf_tensor("T1", [128, W], F32) as T1,
        nc.sbuf_tensor("T2", [128, W], F32) as T2,
        nc.sbuf_tensor("S", [128, W], F32) as S,
        nc.sbuf_tensor("PM", [128, 256], F32) as PM,
        nc.sbuf_tensor("SN", [128, NSNAP * 16], F32) as SN,
        nc.sbuf_tensor("ZZ", [128, 1], F32) as ZZ,
        nc.psum_tensor("PS", [128, 2 * H], F32) as PS,
    ):
        # ghost-column destination view [128, 2, H]: cols [0,H) and [W-H, W)
        from concourse.ap import AP
        ubase = U[:]
        pstep = ubase.ap[0][0]
        ghost_dst = AP(ubase.tensor, 0, [[pstep, 128], [W - H, 2], [1, H]])
        psbase = PS[:]
        ps_step = psbase.ap[0][0]
        ps_src = AP(psbase.tensor, 0, [[ps_step, 128], [H, 2], [1, H]])

        with nc.Block() as block:
            @block.gpsimd
            def _(g):
                g.memset(ZZ[:], 0.0)
                g.dma_start(U[:], x_in[:]).then_inc(dma_sem, 16)
                g.dma_start(PM[:], pm_in[:]).then_inc(dma_sem, 16)

            zbc = ZZ[:].to_broadcast([128, 16])

            @block.vector
            def _(v):
                v.wait_ge(dma_sem, 32)
                # t=0 snapshot.  Snapshots use a 1x-mode tensor_tensor add
                # (a 2x-mode tensor_copy outpaces the previous op's SBUF
                # writeback and reads stale tail columns) plus a small spacer
                # op after the in-place state update.
                v.tensor_add(SN[:, 0:16], U[:, H:H + CH:2], zbc)
                def two_group(tile, off, width):
                    """[128, 2, width] view: cols [off, off+width) and
                    [off + W-H-2, ...) — the two step-1 edge ranges."""
                    base = tile[:]
                    return AP(base.tensor, off,
                              [[base.ap[0][0], 128], [W - H - 2, 2], [1, width]])

                step = 0
                snap = 1
                pending_snap = False
                for blk in range(n_blocks):
                    if blk > 0:
                        # --- step 1, split around the ghost wait ----------
                        # Interior piece reads only core columns (valid before
                        # the exchange lands) and writes scratch, so it hides
                        # under the PE round trip.  Edge piece runs after the
                        # ghost copy.  The in-place state write (un) stays
                        # whole.
                        loI, hiI = H + 1, W - H - 1
                        v.scalar_tensor_tensor(T1[:, loI:hiI], U[:, loI:hiI],
                                               C2, U[:, loI - 1:hiI - 1],
                                               ALU.add, ALU.mult)
                        v.scalar_tensor_tensor(T2[:, loI:hiI], U[:, loI:hiI],
                                               C2, U[:, loI + 1:hiI + 1],
                                               ALU.subtract, ALU.mult)
                        if pending_snap:
                            v.tensor_add(SN[:, snap * 16:snap * 16 + 16],
                                         U[:, H:H + CH:2], zbc)
                            snap += 1
                            pending_snap = False
                        v.tensor_sub(S[:, loI:hiI], T1[:, loI:hiI],
                                     T2[:, loI:hiI])
                        v.wait_ge(p_sem, blk)
                        v.tensor_copy(ghost_dst, ps_src)
                        v.scalar_tensor_tensor(two_group(T1, 1, H),
                                               two_group(U, 1, H), C2,
                                               two_group(U, 0, H),
                                               ALU.add, ALU.mult)
                        v.scalar_tensor_tensor(two_group(T2, 1, H),
                                               two_group(U, 1, H), C2,
                                               two_group(U, 2, H),
                                               ALU.subtract, ALU.mult)
                        v.tensor_sub(two_group(S, 1, H), two_group(T1, 1, H),
                                     two_group(T2, 1, H))
                        v.scalar_tensor_tensor(U[:, 1:W - 1], U[:, 1:W - 1],
                                               LIN, S[:, 1:W - 1],
                                               ALU.mult, ALU.add)
                        v.tensor_sub(S[:, 0:2], T1[:, 0:2], T2[:, 0:2])
                        step += 1
                        if step % SNAP_EVERY == 0:
                            pending_snap = True
                        s_start = 2
                    else:
                        s_start = 1
                    for s in range(s_start, H + 1):
                        lo, hi = s, W - s
                        c = U[:, lo:hi]
                        l = U[:, lo - 1:hi - 1]
                        r = U[:, lo + 1:hi + 1]
                        v.scalar_tensor_tensor(T1[:, lo:hi], c, C2, l,
                                               ALU.add, ALU.mult)
                        v.scalar_tensor_tensor(T2[:, lo:hi], c, C2, r,
                                               ALU.subtract, ALU.mult)
                        if pending_snap:
                            # snapshot of the PREVIOUS step's state: U's core
                            # columns are untouched since then, and the two
                            # STT ops above give the writeback margin.
                            v.tensor_add(SN[:, snap * 16:snap * 16 + 16],
                                         U[:, H:H + CH:2], zbc)
                            snap += 1
                            pending_snap = False
                        v.tensor_sub(S[:, lo:hi], T1[:, lo:hi], T2[:, lo:hi])
                        un = v.scalar_tensor_tensor(c, c, LIN, S[:, lo:hi],
                                                    ALU.mult, ALU.add)
                        # writeback-margin spacer: the next op reads U at the
                        # same streaming rate the in-place update wrote it;
                        # without a gap it can read stale columns.  At block
                        # ends this hides under the exchange stall anyway.
                        v.tensor_sub(S[:, 0:2], T1[:, 0:2], T2[:, 0:2])
                        step += 1
                        if blk < n_blocks - 1 and s == H:
                            un.then_inc(x_sem, 1)
                        if step % SNAP_EVERY == 0:
                            pending_snap = True
                # final snapshot (step == NSTEPS): two spacer ops, then read
                v.tensor_sub(S[:, 0:4], T1[:, 0:4], T2[:, 0:4])
                v.tensor_sub(S[:, 4:8], T1[:, 4:8], T2[:, 4:8])
                v.tensor_add(SN[:, snap * 16:snap * 16 + 16],
                             U[:, H:H + CH:2], zbc).then_inc(v_sem, 1)

            @block.tensor
            def _(t):
                for k in range(1, n_blocks):
                    t.wait_ge(x_sem, k)
                    t.matmul(PS[:, 0:H], PM[:, 0:128], U[:, CH:CH + H],
                             start=True, stop=True)
                    t.matmul(PS[:, H:2 * H], PM[:, 128:256], U[:, H:2 * H],
                             start=True, stop=True).then_inc(p_sem, 1)

            @block.gpsimd
            def _(g):
                g.wait_ge(v_sem, 1)
                g.dma_start(y_out[:], SN[:]).then_inc(dma_sem, 16)
                g.wait_ge(dma_sem, 48)

    return nc


def _perm_inputs():
    """[128, 256] fp32: lhsT_L | lhsT_R permutation matrices.

    out[m,:] = sum_k lhsT[k,m] * rhs[k,:]  ->  lhsT[src(m), m] = 1.
    Left ghosts come from chunk c-1, right ghosts from chunk c+1 (mod 16,
    within the same batch group of 16 partitions).
    """
    pm = np.zeros((128, 256), dtype=np.float32)
    for m in range(128):
        b, c = divmod(m, NCHUNK)
        src_l = b * NCHUNK + (c - 1) % NCHUNK
        src_r = b * NCHUNK + (c + 1) % NCHUNK
        pm[src_l, m] = 1.0
        pm[src_r, 128 + m] = 1.0
    return pm


def _interp_init(u0):
    """Replicate the reference's 1D border-padded linear interp, f32."""
    u0 = np.asarray(u0, dtype=np.float32)
    n_in = u0.shape[1]
    X = np.linspace(0.0, 1.0, MX, dtype=np.float32)
    pts = X * np.float32(2.0) - np.float32(1.0)
    idx = (pts + np.float32(1.0)) * np.float32(0.5) * np.float32(n_in - 1)
    idx = np.clip(idx, 0.0, np.float32(n_in - 1))
    i0 = np.floor(idx).astype(np.int32)
    i0 = np.clip(i0, 0, n_in - 2)
    frac = (idx - i0.astype(np.float32)).astype(np.float32)
    u0f = u0[:, i0] * (np.float32(1.0) - frac) + u0[:, i0 + 1] * frac
    return u0f[:, :-1].astype(np.float32)   # [B, 512]


def kernel(u0):
    from concourse.bass_utils import run_bass_kernel_spmd

    u0 = np.asarray(u0, dtype=np.float32)
    B = u0.shape[0]
    assert B == NCORES * BPC and u0.shape[1] == 512

    u_init = _interp_init(u0)                       # [64, 512]
    w0 = (np.float32(C1) * u_init).astype(np.float32)

    # build per-core input tiles [128, W] with pre-filled ghosts
    cc, xx = np.meshgrid(np.arange(NCHUNK), np.arange(W), indexing="ij")
    src = (cc * CH + xx - H) % 512                  # [16, W]
    pm = _perm_inputs()
    in_maps = []
    for core in range(NCORES):
        wrows = w0[core * BPC:(core + 1) * BPC]     # [8, 512]
        tile = wrows[:, src].astype(np.float32)     # [8, 16, W]
        in_maps.append({"x": tile.reshape(128, W), "pm": pm})

    if "nc" not in _COMPILED:
        _COMPILED["nc"] = _build()
    nc = _COMPILED["nc"]

    res = run_bass_kernel_spmd(nc, in_maps, core_ids=list(range(NCORES)))

    out = np.empty((B, 257, NSNAP), dtype=np.float32)
    inv_c1 = np.float32(1.0 / C1)
    for core in range(NCORES):
        y = res.results[core]["y"]                  # [128, NSNAP*16]
        y = y.reshape(BPC, NCHUNK, NSNAP, 16)       # [b, chunk, t, k]
        u = y * inv_c1
        # spatial index nx = chunk*16 + k  (covers 0..255)
        out[core * BPC:(core + 1) * BPC, 0:256, :] = (
            u.transpose(0, 1, 3, 2).reshape(BPC, 256, NSNAP))
    out[:, 256, :] = out[:, 0, :]
    return out



# revision 3
# speedup vs baseline: 4.3426x; 4.3426x over previous
"""Trainium2 Bass kernel for nn_BurgersSolver_75333726371954.

Burgers' equation explicit solver: interpolate u0 [64,512] to a 513-point
grid, run 5000 sequential periodic-stencil steps on [64,512], snapshot every
50th step at every 2nd spatial point -> [64,257,101].

Strategy (pure data parallel, batch sharded 8 rows/core across 8 cores):
  * Scaled state w = C1*u so the update is
        w' = LIN*w + w_l*(C2+w) + w_r*(C2-w),   LIN = 1-2*C2
  * A custom DVE op (BURGERS_STEP2_ANT) computes one full time step per
    PASS over the stream, using two depth-1 temporal feedback taps
    (CURR_ALU_OUT z^-1 of the input stream for the left term, and z^-1 of
    an intermediate accumulator for combining the right term).  One input
    stream + one output stream; out base = in base - 1.
  * MULTI-PASS instructions: all APs get a stride-0 middle dim [K=10], so
    ONE instruction = 10 time steps (the ~75ns/instruction sequencer
    overhead and the per-pass ~13ns row-restart are the only overheads
    beyond the ~1.05ns/element 1x DVE rate).  K <= 12 (hw row-count limit)
    and stream width >= ~112 elements (the src prefetcher runs ~110
    elements ahead of compute; narrower passes read stale data).
  * Layout [128 partitions = 8 batch x 16 spatial chunks of 32 sites,
    free = 2 dead cols + (H | 32 core | H) + 6 pad].  Ghost depth H=40
    allows 40 steps (4 instructions) between halo refreshes; the valid
    region tapers by 1/side/step.
  * Halo refresh via 4 DVE stream_shuffle copies (partition rotation +-1
    and +-2 within each 16-chunk group -- ghost depth 40 > chunk size 32
    needs two hops).  Everything runs on the vector engine in program
    order -- no PE, no PSUM, no cross-engine semaphores.
  * Snapshots every 50 steps land exactly on instruction boundaries
    (50 = 5 x K); strided 1x tensor-add into an SBUF accumulator, single
    DMA out at the end.  Host rescales by 1/C1 and assembles the output.
"""

import numpy as np

# ---- problem constants (hardcoded; must match the reference config) ----
MX = 513
MT = 5001
DX = 1.0 / (MX - 1)
DT = 1.0 / (MT - 1)
C1 = DT / (2.0 * DX)            # 0.0512
C2 = 0.005 * DT / DX ** 2       # 0.262144
LIN = float(1.0 - 2.0 * C2)

NSTEPS = MT - 1                 # 5000
SNAP_EVERY = 50
NSNAP = NSTEPS // SNAP_EVERY + 1  # 101

NCORES = 8
BPC = 8                         # batch rows per core
NCHUNK = 16                     # spatial chunks per batch row
CH = 32                         # chunk width (NCHUNK*CH == 512)
H = 40                          # ghost depth == steps per block
KP = 10                         # time steps (passes) per DVE instruction
NIPB = H // KP                  # step instructions per block (4)
SW = CH + 2 * H                 # live state width (114)
PAD = 6                         # junk pad to keep the stream comfortably
                                # above the ~112-element staleness floor
WL = SW + PAD                   # instruction stream width (120)
TB = 2                          # dead leading cols (garbage landing zone)
TW = TB + WL                    # tile free width

_COMPILED = {}

# ---------------------------------------------------------------------------
# Custom DVE op: one Burgers step per pass, single input stream.
#
#   w'[c] = LIN*w[c] + w[c-1]*(C2+w[c]) + w[c+1]*(C2-w[c])
#
# Stream x = in0 (base b).  At position t (cur = x[b+t], xp = z^-1):
#   t_a = C2 + cur; t_b = C2 - xp; Q = xp*t_a; P = cur*t_b
#   A = LIN*cur + Q;  out[t] = z^-1(A) + P     -> center c = b+t-1
# out positions 0,1 are garbage (stale taps) and land on taper columns.
# ---------------------------------------------------------------------------

OP_NAME = "BURGERS_STEP2_ANT"


def _stencil_ref(in0, in1, s0, s1, imm2):
    out = np.zeros_like(in0)
    x = in0
    cur, xl, xr = x[:, 1:-1], x[:, :-2], x[:, 2:]
    out[:, 2:] = s0 * cur + xl * (s1 + cur) + xr * (s1 - cur)
    return out


def _build_uops():
    from concourse.dve_uop import (
        ENABLE, AluInp, AluOp, DelayInp, InpSel, OutPath, OutSel, Trigger,
        UopConfig,
    )

    u = UopConfig()
    u.enable_input(InpSel.SRC_0, 1)    # lane1 -> blk0's PREV_DELAY_0 port
    u.enable_input(InpSel.CONST_0, 2)  # LIN   -> PREV_DELAY_1
    u.enable_input(InpSel.CONST_1, 3)  # C2    -> PREV_DELAY_2
    u.require_inp0 = ENABLE
    u.trigger = (Trigger.SRC_TENSOR_DONE, Trigger.NONE, Trigger.NONE)
    u.next_uop = (0, 0, 0)
    u.enable_output(OutSel.ALU_OUT, OutPath.WR0_LO)
    dp = u.datapath_config
    # slice 0: stream -> flop0 (cur); d0 = z^-1 = xp; constants ride d1, d2
    dp[0].enable_alu(AluOp.BYPASS, AluInp.PREV_DELAY_0)
    dp[0].enable_delay_from_src(DelayInp.CURR_ALU_OUT, 0)    # d0 = xp
    dp[0].pass_through_delay(1, 2)
    # slice 1: t_a = cur + C2 ; keep cur in d3
    dp[1].enable_alu(AluOp.ADD, AluInp.PREV_ALU_OUT, AluInp.PREV_DELAY_2)
    dp[1].enable_delay_from_src(DelayInp.PREV_ALU_OUT, 3)    # d3 = cur
    dp[1].pass_through_delay(0, 1, 2)
    # slice 2: t_b = C2 - xp ; keep t_a in d4
    dp[2].enable_alu(AluOp.SUBTRACT, AluInp.PREV_DELAY_2, AluInp.PREV_DELAY_0)
    dp[2].enable_delay_from_src(DelayInp.PREV_ALU_OUT, 4)    # d4 = t_a
    dp[2].pass_through_delay(0, 1, 3)
    # slice 3: Q = xp * t_a ; keep t_b in d5
    dp[3].enable_alu(AluOp.MULTIPLY, AluInp.PREV_DELAY_0, AluInp.PREV_DELAY_4)
    dp[3].enable_delay_from_src(DelayInp.PREV_ALU_OUT, 5)    # d5 = t_b
    dp[3].pass_through_delay(1, 3)
    # slice 4: P = cur * t_b ; keep Q in d0
    dp[4].enable_alu(AluOp.MULTIPLY, AluInp.PREV_DELAY_3, AluInp.PREV_DELAY_5)
    dp[4].enable_delay_from_src(DelayInp.PREV_ALU_OUT, 0)    # d0 = Q
    dp[4].pass_through_delay(1, 3)
    # slice 5: A_lin = LIN * cur ; keep P in d1
    dp[5].enable_alu(AluOp.MULTIPLY, AluInp.PREV_DELAY_1, AluInp.PREV_DELAY_3)
    dp[5].enable_delay_from_src(DelayInp.PREV_ALU_OUT, 1)    # d1 = P
    dp[5].pass_through_delay(0)
    # slice 6: A = A_lin + Q ; d2 = z^-1(A) from own flop
    dp[6].enable_alu(AluOp.ADD, AluInp.PREV_ALU_OUT, AluInp.PREV_DELAY_0)
    dp[6].enable_delay_from_src(DelayInp.CURR_ALU_OUT, 2)    # d2 = A[t-1]
    dp[6].pass_through_delay(1)
    # slice 7: out = z^-1(A) + P
    dp[7].enable_alu(AluOp.ADD, AluInp.PREV_DELAY_2, AluInp.PREV_DELAY_1)
    u.validate("v3")
    return [u]


class _RawDveOp:
    """Duck-typed DveOp whose compile() returns hand-built uops."""

    def __init__(self, name):
        from concourse.dve_spec import Spec, Src0

        self.name = name
        self.subdim = False
        self.perf_en = {}
        self.spec = Spec(body=Src0, reference=_stencil_ref)
        self._compiled = {}

    def compile(self, ver):
        if ver not in self._compiled:
            from concourse.dve_ops import get_dve_sub_opcode
            from concourse.dve_uop import DveOpSpec

            self._compiled[ver] = DveOpSpec(
                name=self.name,
                opcode=get_dve_sub_opcode(self.name),
                uops=_build_uops(),
                rd1_en=False,
            )
        return self._compiled[ver]


def _register_stencil_op():
    import concourse.dve_ops as dve_ops

    for op in dve_ops.OPS:
        if op.name == OP_NAME:
            return op
    op = _RawDveOp(OP_NAME)
    dve_ops.OPS.append(op)
    dve_ops.CUSTOM_DVE_SPECS[OP_NAME] = op.spec
    dve_ops._SUB_OPCODE_FOR_NAME[OP_NAME] = (
        max(dve_ops._SUB_OPCODE_FOR_NAME.values()) + 1
    )
    assert dve_ops._SUB_OPCODE_FOR_NAME[OP_NAME] < 0x20
    return op


def _rot_mask(d):
    """out lane l takes input lane (l%16 + d)%16 within its 16-chunk group."""
    return [(l // 16) * 16 + ((l % 16) + d) % 16 for l in range(32)]


# ghost refresh pieces: (dst_lo, dst_hi, src_lo, rotation)  [state cols]
_GHOSTS = [
    (8, 40, 40, -1),      # left ghosts [8,40)   <- chunk c-1 cores [40,72)
    (0, 8, 64, -2),       # left ghosts [0,8)    <- chunk c-2 cores [64,72)
    (H + CH + 2, H + CH + 32, 42, +1),   # right [74,104) <- c+1 [42,72)
    (H + CH + 32, SW, 40, +2),           # right [104,114) <- c+2 [40,50)
]


def _build():
    import concourse.bass as bass
    import concourse.mybir as mybir
    from concourse.ap import AP

    stencil = _register_stencil_op()

    F32 = mybir.dt.float32

    nc = bass.Bass()
    x_in = nc.dram_tensor("x", [128, TW], F32, kind="ExternalInput")
    y_out = nc.dram_tensor("y", [128, NSNAP * 16], F32, kind="ExternalOutput")

    n_blocks = NSTEPS // H
    assert NSTEPS % H == 0 and H % KP == 0 and SNAP_EVERY % KP == 0

    with (
        nc.semaphore("dma_sem") as dma_sem,
        nc.semaphore("v_sem") as v_sem,
        nc.sbuf_tensor("U", [128, TW], F32) as U,
        nc.sbuf_tensor("SN", [128, NSNAP * 16], F32) as SN,
        nc.sbuf_tensor("ZZ", [128, 1], F32) as ZZ,
        nc.sbuf_tensor("SC", [128, 4], F32) as SC,
    ):
        ub = U[:]
        ps = ub.ap[0][0]
        in0 = AP(ub.tensor, TB, [[ps, 128], [0, KP], [1, WL]])
        out = AP(ub.tensor, TB - 1, [[ps, 128], [0, KP], [1, WL]])

        with nc.Block() as block:
            @block.gpsimd
            def _(g):
                g.memset(ZZ[:], 0.0)
                g.memset(SC[:], 0.0)
                g.dma_start(U[:], x_in[:]).then_inc(dma_sem, 16)

            zbc = ZZ[:].to_broadcast([128, 16])
            zbc2 = ZZ[:].to_broadcast([128, 2])

            @block.vector
            def _(v):
                v.wait_ge(dma_sem, 16)

                def snapshot(k):
                    # 1x tensor_tensor add (not a 2x copy) for writeback margin
                    v.tensor_add(SN[:, k * 16:k * 16 + 16],
                                 U[:, TB + H:TB + H + CH:2], zbc)

                snapshot(0)
                snap = 1
                step = 0
                for blk in range(n_blocks):
                    if blk > 0:
                        for dlo, dhi, slo, rot in _GHOSTS:
                            w = dhi - dlo
                            v.stream_shuffle(U[:, TB + dlo:TB + dhi],
                                             U[:, TB + slo:TB + slo + w],
                                             _rot_mask(rot))
                        # writeback-margin spacer before the step instruction
                        # prefetches the freshly shuffled ghost columns
                        v.tensor_add(SC[:, 0:2], SC[:, 2:4], zbc2)
                    for i in range(NIPB):
                        v._custom_dve(stencil, out=out, in0=in0,
                                      s0=LIN, s1=C2)
                        step += KP
                        if step % SNAP_EVERY == 0 and i < NIPB - 1:
                            snapshot(snap)
                            snap += 1
                    if step % SNAP_EVERY == 0:
                        snapshot(snap)
                        snap += 1
                assert snap == NSNAP, snap
                v.tensor_add(SC[:, 0:2], SC[:, 2:4], zbc2).then_inc(v_sem, 1)

            @block.gpsimd
            def _(g):
                g.wait_ge(v_sem, 1)
                g.dma_start(y_out[:], SN[:]).then_inc(dma_sem, 16)
                g.wait_ge(dma_sem, 32)

    mybir.codegen_inst_isa_subclasses(nc)
    return nc


def _interp_init(u0):
    """Replicate the reference's 1D border-padded linear interp, f32."""
    u0 = np.asarray(u0, dtype=np.float32)
    n_in = u0.shape[1]
    X = np.linspace(0.0, 1.0, MX, dtype=np.float32)
    pts = X * np.float32(2.0) - np.float32(1.0)
    idx = (pts + np.float32(1.0)) * np.float32(0.5) * np.float32(n_in - 1)
    idx = np.clip(idx, 0.0, np.float32(n_in - 1))
    i0 = np.floor(idx).astype(np.int32)
    i0 = np.clip(i0, 0, n_in - 2)
    frac = (idx - i0.astype(np.float32)).astype(np.float32)
    u0f = u0[:, i0] * (np.float32(1.0) - frac) + u0[:, i0 + 1] * frac
    return u0f[:, :-1].astype(np.float32)   # [B, 512]


def _in_maps(u0):
    """Per-core input tiles [128, TW]: dead cols + prefilled ghosts + pad."""
    u_init = _interp_init(u0)                       # [64, 512]
    w0 = (np.float32(C1) * u_init).astype(np.float32)
    cc, xx = np.meshgrid(np.arange(NCHUNK), np.arange(TW), indexing="ij")
    src = (cc * CH + xx - TB - H) % 512             # [16, TW]
    maps = []
    for core in range(NCORES):
        wrows = w0[core * BPC:(core + 1) * BPC]     # [8, 512]
        tile = wrows[:, src].astype(np.float32)     # [8, 16, TW]
        maps.append({"x": tile.reshape(128, TW)})
    return maps


def kernel(u0):
    from concourse.bass_utils import run_bass_kernel_spmd

    u0 = np.asarray(u0, dtype=np.float32)
    B = u0.shape[0]
    assert B == NCORES * BPC and u0.shape[1] == 512

    if "nc" not in _COMPILED:
        _COMPILED["nc"] = _build()
    nc = _COMPILED["nc"]

    res = run_bass_kernel_spmd(nc, _in_maps(u0), core_ids=list(range(NCORES)))

    out = np.empty((B, 257, NSNAP), dtype=np.float32)
    inv_c1 = np.float32(1.0 / C1)
    for core in range(NCORES):
        y = res.results[core]["y"]                  # [128, NSNAP*16]
        y = y.reshape(BPC, NCHUNK, NSNAP, 16)       # [b, chunk, t, k]
        u = y * inv_c1
        # spatial index nx = chunk*16 + k  (covers 0..255)
        out[core * BPC:(core + 1) * BPC, 0:256, :] = (
            u.transpose(0, 1, 3, 2).reshape(BPC, 256, NSNAP))
    out[:, 256, :] = out[:, 0, :]
    return out


# revision 9
# speedup vs baseline: 4.4367x; 1.0217x over previous
"""Trainium2 Bass kernel for nn_BurgersSolver_75333726371954.

Burgers' equation explicit solver: interpolate u0 [64,512] to a 513-point
grid, run 5000 sequential periodic-stencil steps on [64,512], snapshot every
50th step at every 2nd spatial point -> [64,257,101].

Strategy (pure data parallel, batch sharded 8 rows/core across 8 cores):
  * Scaled state w = C1*u so the update is
        w' = LIN*w + w_l*(C2+w) + w_r*(C2-w),   LIN = 1-2*C2
  * A custom DVE op (BURGERS_STEP2_ANT) computes one full time step per
    PASS over the stream, using two depth-1 temporal feedback taps
    (CURR_ALU_OUT z^-1 of the input stream for the left term, and z^-1 of
    an intermediate accumulator for combining the right term).  One input
    stream + one output stream; out base = in base - 1.
  * MULTI-PASS instructions: all APs get a stride-0 middle dim [K=10], so
    ONE instruction = 10 time steps (the ~75ns/instruction sequencer
    overhead and the per-pass ~13ns row-restart are the only overheads
    beyond the ~1.05ns/element 1x DVE rate).  K <= 12 (hw row-count limit)
    and stream width >= ~112 elements (the src prefetcher runs ~110
    elements ahead of compute; narrower passes read stale data).
  * Layout [128 partitions = 8 batch x 16 spatial chunks of 32 sites,
    free = 2 dead cols + (H | 32 core | H) + 6 pad].  Ghost depth H=40
    allows 40 steps (4 instructions) between halo refreshes; the valid
    region tapers by 1/side/step.
  * Halo refresh via 4 DVE stream_shuffle copies (partition rotation +-1
    and +-2 within each 16-chunk group -- ghost depth 40 > chunk size 32
    needs two hops).  Everything runs on the vector engine in program
    order -- no PE, no PSUM, no cross-engine semaphores.
  * Snapshots every 50 steps land exactly on instruction boundaries
    (50 = 5 x K); strided 1x tensor-add into an SBUF accumulator, single
    DMA out at the end.  Host rescales by 1/C1 and assembles the output.
"""

import numpy as np

# ---- problem constants (hardcoded; must match the reference config) ----
MX = 513
MT = 5001
DX = 1.0 / (MX - 1)
DT = 1.0 / (MT - 1)
C1 = DT / (2.0 * DX)            # 0.0512
C2 = 0.005 * DT / DX ** 2       # 0.262144
LIN = float(1.0 - 2.0 * C2)

NSTEPS = MT - 1                 # 5000
SNAP_EVERY = 50
NSNAP = NSTEPS // SNAP_EVERY + 1  # 101

NCORES = 8
BPC = 8                         # batch rows per core
NCHUNK = 16                     # spatial chunks per batch row
CH = 32                         # chunk width (NCHUNK*CH == 512)
H = 25                          # ghost depth == steps per block
KSPLIT = (10, 10, 5)            # passes per DVE instruction (sum == H; <= 12)
SW = CH + 2 * H                 # live state width (84)
PAD = 32                        # junk pad to keep the stream comfortably
                                # above the ~112-element staleness floor
WL = SW + PAD                   # instruction stream width (114)
TB = 2                          # dead leading cols (garbage landing zone)
TW = TB + WL                    # tile free width

_COMPILED = {}

# ---------------------------------------------------------------------------
# Custom DVE op: one Burgers step per pass, single input stream.
#
#   w'[c] = LIN*w[c] + w[c-1]*(C2+w[c]) + w[c+1]*(C2-w[c])
#
# Stream x = in0 (base b).  At position t (cur = x[b+t], xp = z^-1):
#   t_a = C2 + cur; t_b = C2 - xp; Q = xp*t_a; P = cur*t_b
#   A = LIN*cur + Q;  out[t] = z^-1(A) + P     -> center c = b+t-1
# out positions 0,1 are garbage (stale taps) and land on taper columns.
# ---------------------------------------------------------------------------

OP_NAME = "BURGERS_STEP2_ANT"


def _stencil_ref(in0, in1, s0, s1, imm2):
    out = np.zeros_like(in0)
    x = in0
    cur, xl, xr = x[:, 1:-1], x[:, :-2], x[:, 2:]
    out[:, 2:] = s0 * cur + xl * (s1 + cur) + xr * (s1 - cur)
    return out


def _build_uops():
    from concourse.dve_uop import (
        ENABLE, AluInp, AluOp, DelayInp, InpSel, OutPath, OutSel, Trigger,
        UopConfig,
    )

    u = UopConfig()
    u.enable_input(InpSel.SRC_0, 1)    # lane1 -> blk0's PREV_DELAY_0 port
    u.enable_input(InpSel.CONST_0, 2)  # LIN   -> PREV_DELAY_1
    u.enable_input(InpSel.CONST_1, 3)  # C2    -> PREV_DELAY_2
    u.require_inp0 = ENABLE
    u.trigger = (Trigger.SRC_TENSOR_DONE, Trigger.NONE, Trigger.NONE)
    u.next_uop = (0, 0, 0)
    u.enable_output(OutSel.ALU_OUT, OutPath.WR0_LO)
    dp = u.datapath_config
    # slice 0: stream -> flop0 (cur); d0 = z^-1 = xp; constants ride d1, d2
    dp[0].enable_alu(AluOp.BYPASS, AluInp.PREV_DELAY_0)
    dp[0].enable_delay_from_src(DelayInp.CURR_ALU_OUT, 0)    # d0 = xp
    dp[0].pass_through_delay(1, 2)
    # slice 1: t_a = cur + C2 ; keep cur in d3
    dp[1].enable_alu(AluOp.ADD, AluInp.PREV_ALU_OUT, AluInp.PREV_DELAY_2)
    dp[1].enable_delay_from_src(DelayInp.PREV_ALU_OUT, 3)    # d3 = cur
    dp[1].pass_through_delay(0, 1, 2)
    # slice 2: t_b = C2 - xp ; keep t_a in d4
    dp[2].enable_alu(AluOp.SUBTRACT, AluInp.PREV_DELAY_2, AluInp.PREV_DELAY_0)
    dp[2].enable_delay_from_src(DelayInp.PREV_ALU_OUT, 4)    # d4 = t_a
    dp[2].pass_through_delay(0, 1, 3)
    # slice 3: Q = xp * t_a ; keep t_b in d5
    dp[3].enable_alu(AluOp.MULTIPLY, AluInp.PREV_DELAY_0, AluInp.PREV_DELAY_4)
    dp[3].enable_delay_from_src(DelayInp.PREV_ALU_OUT, 5)    # d5 = t_b
    dp[3].pass_through_delay(1, 3)
    # slice 4: P = cur * t_b ; keep Q in d0
    dp[4].enable_alu(AluOp.MULTIPLY, AluInp.PREV_DELAY_3, AluInp.PREV_DELAY_5)
    dp[4].enable_delay_from_src(DelayInp.PREV_ALU_OUT, 0)    # d0 = Q
    dp[4].pass_through_delay(1, 3)
    # slice 5: A_lin = LIN * cur ; keep P in d1
    dp[5].enable_alu(AluOp.MULTIPLY, AluInp.PREV_DELAY_1, AluInp.PREV_DELAY_3)
    dp[5].enable_delay_from_src(DelayInp.PREV_ALU_OUT, 1)    # d1 = P
    dp[5].pass_through_delay(0)
    # slice 6: A = A_lin + Q ; d2 = z^-1(A) from own flop
    dp[6].enable_alu(AluOp.ADD, AluInp.PREV_ALU_OUT, AluInp.PREV_DELAY_0)
    dp[6].enable_delay_from_src(DelayInp.CURR_ALU_OUT, 2)    # d2 = A[t-1]
    dp[6].pass_through_delay(1)
    # slice 7: out = z^-1(A) + P
    dp[7].enable_alu(AluOp.ADD, AluInp.PREV_DELAY_2, AluInp.PREV_DELAY_1)
    u.validate("v3")
    return [u]


class _RawDveOp:
    """Duck-typed DveOp whose compile() returns hand-built uops."""

    def __init__(self, name):
        from concourse.dve_spec import Spec, Src0

        self.name = name
        self.subdim = False
        self.perf_en = {}
        self.spec = Spec(body=Src0, reference=_stencil_ref)
        self._compiled = {}

    def compile(self, ver):
        if ver not in self._compiled:
            from concourse.dve_ops import get_dve_sub_opcode
            from concourse.dve_uop import DveOpSpec

            self._compiled[ver] = DveOpSpec(
                name=self.name,
                opcode=get_dve_sub_opcode(self.name),
                uops=_build_uops(),
                rd1_en=False,
            )
        return self._compiled[ver]


def _register_stencil_op():
    import concourse.dve_ops as dve_ops

    for op in dve_ops.OPS:
        if op.name == OP_NAME:
            return op
    op = _RawDveOp(OP_NAME)
    dve_ops.OPS.append(op)
    dve_ops.CUSTOM_DVE_SPECS[OP_NAME] = op.spec
    dve_ops._SUB_OPCODE_FOR_NAME[OP_NAME] = (
        max(dve_ops._SUB_OPCODE_FOR_NAME.values()) + 1
    )
    assert dve_ops._SUB_OPCODE_FOR_NAME[OP_NAME] < 0x20
    return op


def _rot_mask(d):
    """out lane l takes input lane (l%16 + d)%16 within its 16-chunk group."""
    return [(l // 16) * 16 + ((l % 16) + d) % 16 for l in range(32)]


# ghost refresh pieces: (dst_lo, dst_hi, src_lo, rotation)  [state cols]
# left ghosts [0,H)  <- chunk c-1 cores [CH, CH+H) = state [2H-... wait:
#   ghost col g in [0,H): site offset g-H -> chunk c-1 col g-H+CH
#   = state col g-H+CH+H = g+CH
# right ghosts [H+CH, SW) <- chunk c+1 cols [0, H) = state cols [H, 2H)
_GHOSTS = [
    (0, H, CH, -1),               # left  [0,25)  <- c-1 state [32,57)
    (H + CH, SW, H, +1),          # right [57,84) <- c+1 state [25,50)
]


def _build():
    import concourse.bass as bass
    import concourse.mybir as mybir
    from concourse.ap import AP

    stencil = _register_stencil_op()

    F32 = mybir.dt.float32

    nc = bass.Bass()
    x_in = nc.dram_tensor("x", [128, TW], F32, kind="ExternalInput")
    y_out = nc.dram_tensor("y", [128, NSNAP * 16], F32, kind="ExternalOutput")

    n_blocks = NSTEPS // H
    assert NSTEPS % H == 0 and sum(KSPLIT) == H and SNAP_EVERY % H == 0

    with (
        nc.semaphore("dma_sem") as dma_sem,
        nc.semaphore("v_sem") as v_sem,
        nc.sbuf_tensor("U", [128, TW], F32) as U,
        nc.sbuf_tensor("SN", [128, NSNAP * 16], F32) as SN,
        nc.sbuf_tensor("ZZ", [128, 1], F32) as ZZ,
        nc.sbuf_tensor("SC", [128, 4], F32) as SC,
    ):
        ub = U[:]
        ps = ub.ap[0][0]
        aps = {k: (AP(ub.tensor, TB, [[ps, 128], [0, k], [1, WL]]),
                   AP(ub.tensor, TB - 1, [[ps, 128], [0, k], [1, WL]]))
               for k in set(KSPLIT)}

        with nc.Block() as block:
            @block.gpsimd
            def _(g):
                g.memset(ZZ[:], 0.0)
                g.memset(SC[:], 0.0)
                g.dma_start(U[:], x_in[:]).then_inc(dma_sem, 16)

            zbc = ZZ[:].to_broadcast([128, 16])
            zbc2 = ZZ[:].to_broadcast([128, 2])

            @block.vector
            def _(v):
                v.wait_ge(dma_sem, 16)

                def snapshot(k):
                    # 1x tensor_tensor add (not a 2x copy) for writeback margin
                    v.tensor_add(SN[:, k * 16:k * 16 + 16],
                                 U[:, TB + H:TB + H + CH:2], zbc)

                snapshot(0)
                snap = 1
                step = 0
                for blk in range(n_blocks):
                    if blk > 0:
                        for dlo, dhi, slo, rot in _GHOSTS:
                            w = dhi - dlo
                            v.stream_shuffle(U[:, TB + dlo:TB + dhi],
                                             U[:, TB + slo:TB + slo + w],
                                             _rot_mask(rot))
                        # writeback-margin spacer before the step instruction
                        # prefetches the freshly shuffled ghost columns
                        v.tensor_add(SC[:, 0:2], SC[:, 2:4], zbc2)
                    for k in KSPLIT:
                        in0, out = aps[k]
                        v._custom_dve(stencil, out=out, in0=in0,
                                      s0=LIN, s1=C2)
                        step += k
                    if step % SNAP_EVERY == 0:
                        snapshot(snap)
                        snap += 1
                assert snap == NSNAP, snap
                v.tensor_add(SC[:, 0:2], SC[:, 2:4], zbc2).then_inc(v_sem, 1)

            @block.gpsimd
            def _(g):
                g.wait_ge(v_sem, 1)
                g.dma_start(y_out[:], SN[:]).then_inc(dma_sem, 16)
                g.wait_ge(dma_sem, 32)

    mybir.codegen_inst_isa_subclasses(nc)
    return nc


def _interp_init(u0):
    """Replicate the reference's 1D border-padded linear interp, f32."""
    u0 = np.asarray(u0, dtype=np.float32)
    n_in = u0.shape[1]
    X = np.linspace(0.0, 1.0, MX, dtype=np.float32)
    pts = X * np.float32(2.0) - np.float32(1.0)
    idx = (pts + np.float32(1.0)) * np.float32(0.5) * np.float32(n_in - 1)
    idx = np.clip(idx, 0.0, np.float32(n_in - 1))
    i0 = np.floor(idx).astype(np.int32)
    i0 = np.clip(i0, 0, n_in - 2)
    frac = (idx - i0.astype(np.float32)).astype(np.float32)
    u0f = u0[:, i0] * (np.float32(1.0) - frac) + u0[:, i0 + 1] * frac
    return u0f[:, :-1].astype(np.float32)   # [B, 512]


def _in_maps(u0):
    """Per-core input tiles [128, TW]: dead cols + prefilled ghosts + pad."""
    u_init = _interp_init(u0)                       # [64, 512]
    w0 = (np.float32(C1) * u_init).astype(np.float32)
    cc, xx = np.meshgrid(np.arange(NCHUNK), np.arange(TW), indexing="ij")
    src = (cc * CH + xx - TB - H) % 512             # [16, TW]
    maps = []
    for core in range(NCORES):
        wrows = w0[core * BPC:(core + 1) * BPC]     # [8, 512]
        tile = wrows[:, src].astype(np.float32)     # [8, 16, TW]
        maps.append({"x": tile.reshape(128, TW)})
    return maps


def kernel(u0):
    from concourse.bass_utils import run_bass_kernel_spmd

    u0 = np.asarray(u0, dtype=np.float32)
    B = u0.shape[0]
    assert B == NCORES * BPC and u0.shape[1] == 512

    if "nc" not in _COMPILED:
        _COMPILED["nc"] = _build()
    nc = _COMPILED["nc"]

    res = run_bass_kernel_spmd(nc, _in_maps(u0), core_ids=list(range(NCORES)))

    out = np.empty((B, 257, NSNAP), dtype=np.float32)
    inv_c1 = np.float32(1.0 / C1)
    for core in range(NCORES):
        y = res.results[core]["y"]                  # [128, NSNAP*16]
        y = y.reshape(BPC, NCHUNK, NSNAP, 16)       # [b, chunk, t, k]
        u = y * inv_c1
        # spatial index nx = chunk*16 + k  (covers 0..255)
        out[core * BPC:(core + 1) * BPC, 0:256, :] = (
            u.transpose(0, 1, 3, 2).reshape(BPC, 256, NSNAP))
    out[:, 256, :] = out[:, 0, :]
    return out


# revision 11
# speedup vs baseline: 4.5627x; 1.0284x over previous
"""Trainium2 Bass kernel for nn_BurgersSolver_75333726371954.

Burgers' equation explicit solver: interpolate u0 [64,512] to a 513-point
grid, run 5000 sequential periodic-stencil steps on [64,512], snapshot every
50th step at every 2nd spatial point -> [64,257,101].

Strategy (pure data parallel, batch sharded 8 rows/core across 8 cores):
  * Scaled state w = C1*u so the update is
        w' = LIN*w + w_l*(C2+w) + w_r*(C2-w),   LIN = 1-2*C2
  * A custom DVE op (BURGERS_STEP2_ANT) computes one full time step per
    PASS over the stream, using two depth-1 temporal feedback taps
    (CURR_ALU_OUT z^-1 of the input stream for the left term, and z^-1 of
    an intermediate accumulator for combining the right term).  One input
    stream + one output stream; out base = in base - 1.
  * MULTI-PASS instructions: all APs get a stride-0 middle dim [K=10], so
    ONE instruction = 10 time steps (the ~75ns/instruction sequencer
    overhead and the per-pass ~13ns row-restart are the only overheads
    beyond the ~1.05ns/element 1x DVE rate).  K <= 12 (hw row-count limit)
    and stream width >= ~112 elements (the src prefetcher runs ~110
    elements ahead of compute; narrower passes read stale data).
  * Layout [128 partitions = 8 batch x 16 spatial chunks of 32 sites,
    free = 2 dead cols + (H | 32 core | H) + 6 pad].  Ghost depth H=40
    allows 40 steps (4 instructions) between halo refreshes; the valid
    region tapers by 1/side/step.
  * Halo refresh via 4 DVE stream_shuffle copies (partition rotation +-1
    and +-2 within each 16-chunk group -- ghost depth 40 > chunk size 32
    needs two hops).  Everything runs on the vector engine in program
    order -- no PE, no PSUM, no cross-engine semaphores.
  * Snapshots every 50 steps land exactly on instruction boundaries
    (50 = 5 x K); strided 1x tensor-add into an SBUF accumulator, single
    DMA out at the end.  Host rescales by 1/C1 and assembles the output.
"""

import numpy as np

# ---- problem constants (hardcoded; must match the reference config) ----
MX = 513
MT = 5001
DX = 1.0 / (MX - 1)
DT = 1.0 / (MT - 1)
C1 = DT / (2.0 * DX)            # 0.0512
C2 = 0.005 * DT / DX ** 2       # 0.262144
LIN = float(1.0 - 2.0 * C2)

NSTEPS = MT - 1                 # 5000
SNAP_EVERY = 50
NSNAP = NSTEPS // SNAP_EVERY + 1  # 101

NCORES = 8
BPC = 8                         # batch rows per core
NCHUNK = 16                     # spatial chunks per batch row
CH = 32                         # chunk width (NCHUNK*CH == 512)
H = 25                          # ghost depth == steps per block
KSPLIT = (12, 13)               # passes per DVE instruction (sum == H; <= 13)
SW = CH + 2 * H                 # live state width (84)
PAD = 32                        # junk pad to keep the stream comfortably
                                # above the ~112-element staleness floor
WL = SW + PAD                   # instruction stream width (114)
TB = 2                          # dead leading cols (garbage landing zone)
TW = TB + WL                    # tile free width

_COMPILED = {}

# ---------------------------------------------------------------------------
# Custom DVE op: one Burgers step per pass, single input stream.
#
#   w'[c] = LIN*w[c] + w[c-1]*(C2+w[c]) + w[c+1]*(C2-w[c])
#
# Stream x = in0 (base b).  At position t (cur = x[b+t], xp = z^-1):
#   t_a = C2 + cur; t_b = C2 - xp; Q = xp*t_a; P = cur*t_b
#   A = LIN*cur + Q;  out[t] = z^-1(A) + P     -> center c = b+t-1
# out positions 0,1 are garbage (stale taps) and land on taper columns.
# ---------------------------------------------------------------------------

OP_NAME = "BURGERS_STEP2_ANT"


def _stencil_ref(in0, in1, s0, s1, imm2):
    out = np.zeros_like(in0)
    x = in0
    cur, xl, xr = x[:, 1:-1], x[:, :-2], x[:, 2:]
    out[:, 2:] = s0 * cur + xl * (s1 + cur) + xr * (s1 - cur)
    return out


def _build_uops():
    from concourse.dve_uop import (
        ENABLE, AluInp, AluOp, DelayInp, InpSel, OutPath, OutSel, Trigger,
        UopConfig,
    )

    u = UopConfig()
    u.enable_input(InpSel.SRC_0, 1)    # lane1 -> blk0's PREV_DELAY_0 port
    u.enable_input(InpSel.CONST_0, 2)  # LIN   -> PREV_DELAY_1
    u.enable_input(InpSel.CONST_1, 3)  # C2    -> PREV_DELAY_2
    u.require_inp0 = ENABLE
    u.trigger = (Trigger.SRC_TENSOR_DONE, Trigger.NONE, Trigger.NONE)
    u.next_uop = (0, 0, 0)
    u.enable_output(OutSel.ALU_OUT, OutPath.WR0_LO)
    dp = u.datapath_config
    # slice 0: stream -> flop0 (cur); d0 = z^-1 = xp; constants ride d1, d2
    dp[0].enable_alu(AluOp.BYPASS, AluInp.PREV_DELAY_0)
    dp[0].enable_delay_from_src(DelayInp.CURR_ALU_OUT, 0)    # d0 = xp
    dp[0].pass_through_delay(1, 2)
    # slice 1: t_a = cur + C2 ; keep cur in d3
    dp[1].enable_alu(AluOp.ADD, AluInp.PREV_ALU_OUT, AluInp.PREV_DELAY_2)
    dp[1].enable_delay_from_src(DelayInp.PREV_ALU_OUT, 3)    # d3 = cur
    dp[1].pass_through_delay(0, 1, 2)
    # slice 2: t_b = C2 - xp ; keep t_a in d4
    dp[2].enable_alu(AluOp.SUBTRACT, AluInp.PREV_DELAY_2, AluInp.PREV_DELAY_0)
    dp[2].enable_delay_from_src(DelayInp.PREV_ALU_OUT, 4)    # d4 = t_a
    dp[2].pass_through_delay(0, 1, 3)
    # slice 3: Q = xp * t_a ; keep t_b in d5
    dp[3].enable_alu(AluOp.MULTIPLY, AluInp.PREV_DELAY_0, AluInp.PREV_DELAY_4)
    dp[3].enable_delay_from_src(DelayInp.PREV_ALU_OUT, 5)    # d5 = t_b
    dp[3].pass_through_delay(1, 3)
    # slice 4: P = cur * t_b ; keep Q in d0
    dp[4].enable_alu(AluOp.MULTIPLY, AluInp.PREV_DELAY_3, AluInp.PREV_DELAY_5)
    dp[4].enable_delay_from_src(DelayInp.PREV_ALU_OUT, 0)    # d0 = Q
    dp[4].pass_through_delay(1, 3)
    # slice 5: A_lin = LIN * cur ; keep P in d1
    dp[5].enable_alu(AluOp.MULTIPLY, AluInp.PREV_DELAY_1, AluInp.PREV_DELAY_3)
    dp[5].enable_delay_from_src(DelayInp.PREV_ALU_OUT, 1)    # d1 = P
    dp[5].pass_through_delay(0)
    # slice 6: A = A_lin + Q ; d2 = z^-1(A) from own flop
    dp[6].enable_alu(AluOp.ADD, AluInp.PREV_ALU_OUT, AluInp.PREV_DELAY_0)
    dp[6].enable_delay_from_src(DelayInp.CURR_ALU_OUT, 2)    # d2 = A[t-1]
    dp[6].pass_through_delay(1)
    # slice 7: out = z^-1(A) + P
    dp[7].enable_alu(AluOp.ADD, AluInp.PREV_DELAY_2, AluInp.PREV_DELAY_1)
    u.validate("v3")
    return [u]


class _RawDveOp:
    """Duck-typed DveOp whose compile() returns hand-built uops."""

    def __init__(self, name):
        from concourse.dve_spec import Spec, Src0

        self.name = name
        self.subdim = False
        self.perf_en = {}
        self.spec = Spec(body=Src0, reference=_stencil_ref)
        self._compiled = {}

    def compile(self, ver):
        if ver not in self._compiled:
            from concourse.dve_ops import get_dve_sub_opcode
            from concourse.dve_uop import DveOpSpec

            self._compiled[ver] = DveOpSpec(
                name=self.name,
                opcode=get_dve_sub_opcode(self.name),
                uops=_build_uops(),
                rd1_en=False,
            )
        return self._compiled[ver]


def _register_stencil_op():
    import concourse.dve_ops as dve_ops

    for op in dve_ops.OPS:
        if op.name == OP_NAME:
            return op
    op = _RawDveOp(OP_NAME)
    dve_ops.OPS.append(op)
    dve_ops.CUSTOM_DVE_SPECS[OP_NAME] = op.spec
    dve_ops._SUB_OPCODE_FOR_NAME[OP_NAME] = (
        max(dve_ops._SUB_OPCODE_FOR_NAME.values()) + 1
    )
    assert dve_ops._SUB_OPCODE_FOR_NAME[OP_NAME] < 0x20
    return op


def _rot_mask(d):
    """out lane l takes input lane (l%16 + d)%16 within its 16-chunk group."""
    return [(l // 16) * 16 + ((l % 16) + d) % 16 for l in range(32)]


# ghost refresh pieces: (dst_lo, dst_hi, src_lo, rotation)  [state cols]
# left ghosts [0,H)  <- chunk c-1 cores [CH, CH+H) = state [2H-... wait:
#   ghost col g in [0,H): site offset g-H -> chunk c-1 col g-H+CH
#   = state col g-H+CH+H = g+CH
# right ghosts [H+CH, SW) <- chunk c+1 cols [0, H) = state cols [H, 2H)
_GHOSTS = [
    (0, H, CH, -1),               # left  [0,25)  <- c-1 state [32,57)
    (H + CH, SW, H, +1),          # right [57,84) <- c+1 state [25,50)
]


def _build():
    import concourse.bass as bass
    import concourse.mybir as mybir
    from concourse.ap import AP

    stencil = _register_stencil_op()

    F32 = mybir.dt.float32

    nc = bass.Bass()
    x_in = nc.dram_tensor("x", [128, TW], F32, kind="ExternalInput")
    y_out = nc.dram_tensor("y", [128, NSNAP * 16], F32, kind="ExternalOutput")

    n_blocks = NSTEPS // H
    assert NSTEPS % H == 0 and sum(KSPLIT) == H and SNAP_EVERY % H == 0

    with (
        nc.semaphore("dma_sem") as dma_sem,
        nc.semaphore("v_sem") as v_sem,
        nc.sbuf_tensor("U", [128, TW], F32) as U,
        nc.sbuf_tensor("SN", [128, NSNAP * 16], F32) as SN,
        nc.sbuf_tensor("ZZ", [128, 1], F32) as ZZ,
        nc.sbuf_tensor("SC", [128, 4], F32) as SC,
    ):
        ub = U[:]
        ps = ub.ap[0][0]
        aps = {k: (AP(ub.tensor, TB, [[ps, 128], [0, k], [1, WL]]),
                   AP(ub.tensor, TB - 1, [[ps, 128], [0, k], [1, WL]]))
               for k in set(KSPLIT)}

        with nc.Block() as block:
            @block.gpsimd
            def _(g):
                g.memset(ZZ[:], 0.0)
                g.memset(SC[:], 0.0)
                g.dma_start(U[:], x_in[:]).then_inc(dma_sem, 16)

            zbc = ZZ[:].to_broadcast([128, 16])
            zbc2 = ZZ[:].to_broadcast([128, 2])

            @block.vector
            def _(v):
                v.wait_ge(dma_sem, 16)

                def snapshot(k):
                    # 1x tensor_tensor add (not a 2x copy) for writeback margin
                    v.tensor_add(SN[:, k * 16:k * 16 + 16],
                                 U[:, TB + H:TB + H + CH:2], zbc)

                snapshot(0)
                snap = 1
                step = 0
                for blk in range(n_blocks):
                    for k in KSPLIT:
                        in0, out = aps[k]
                        v._custom_dve(stencil, out=out, in0=in0,
                                      s0=LIN, s1=C2)
                        step += k
                    if blk < n_blocks - 1:
                        for dlo, dhi, slo, rot in _GHOSTS:
                            w = dhi - dlo
                            v.stream_shuffle(U[:, TB + dlo:TB + dhi],
                                             U[:, TB + slo:TB + slo + w],
                                             _rot_mask(rot))
                    # snapshot doubles as the writeback-margin spacer between
                    # the ghost shuffles and the next step instruction's
                    # prefetch; on non-snapshot blocks use a dummy spacer
                    if step % SNAP_EVERY == 0:
                        snapshot(snap)
                        snap += 1
                    elif blk < n_blocks - 1:
                        v.tensor_add(SC[:, 0:2], SC[:, 2:4], zbc2)
                assert snap == NSNAP, snap
                v.tensor_add(SC[:, 0:2], SC[:, 2:4], zbc2).then_inc(v_sem, 1)

            @block.gpsimd
            def _(g):
                g.wait_ge(v_sem, 1)
                g.dma_start(y_out[:], SN[:]).then_inc(dma_sem, 16)
                g.wait_ge(dma_sem, 32)

    mybir.codegen_inst_isa_subclasses(nc)
    return nc


def _interp_init(u0):
    """Replicate the reference's 1D border-padded linear interp, f32."""
    u0 = np.asarray(u0, dtype=np.float32)
    n_in = u0.shape[1]
    X = np.linspace(0.0, 1.0, MX, dtype=np.float32)
    pts = X * np.float32(2.0) - np.float32(1.0)
    idx = (pts + np.float32(1.0)) * np.float32(0.5) * np.float32(n_in - 1)
    idx = np.clip(idx, 0.0, np.float32(n_in - 1))
    i0 = np.floor(idx).astype(np.int32)
    i0 = np.clip(i0, 0, n_in - 2)
    frac = (idx - i0.astype(np.float32)).astype(np.float32)
    u0f = u0[:, i0] * (np.float32(1.0) - frac) + u0[:, i0 + 1] * frac
    return u0f[:, :-1].astype(np.float32)   # [B, 512]


def _in_maps(u0):
    """Per-core input tiles [128, TW]: dead cols + prefilled ghosts + pad."""
    u_init = _interp_init(u0)                       # [64, 512]
    w0 = (np.float32(C1) * u_init).astype(np.float32)
    cc, xx = np.meshgrid(np.arange(NCHUNK), np.arange(TW), indexing="ij")
    src = (cc * CH + xx - TB - H) % 512             # [16, TW]
    maps = []
    for core in range(NCORES):
        wrows = w0[core * BPC:(core + 1) * BPC]     # [8, 512]
        tile = wrows[:, src].astype(np.float32)     # [8, 16, TW]
        maps.append({"x": tile.reshape(128, TW)})
    return maps


def kernel(u0):
    from concourse.bass_utils import run_bass_kernel_spmd

    u0 = np.asarray(u0, dtype=np.float32)
    B = u0.shape[0]
    assert B == NCORES * BPC and u0.shape[1] == 512

    if "nc" not in _COMPILED:
        _COMPILED["nc"] = _build()
    nc = _COMPILED["nc"]

    res = run_bass_kernel_spmd(nc, _in_maps(u0), core_ids=list(range(NCORES)))

    out = np.empty((B, 257, NSNAP), dtype=np.float32)
    inv_c1 = np.float32(1.0 / C1)
    for core in range(NCORES):
        y = res.results[core]["y"]                  # [128, NSNAP*16]
        y = y.reshape(BPC, NCHUNK, NSNAP, 16)       # [b, chunk, t, k]
        u = y * inv_c1
        # spatial index nx = chunk*16 + k  (covers 0..255)
        out[core * BPC:(core + 1) * BPC, 0:256, :] = (
            u.transpose(0, 1, 3, 2).reshape(BPC, 256, NSNAP))
    out[:, 256, :] = out[:, 0, :]
    return out


# revision 12
# speedup vs baseline: 4.6915x; 1.0282x over previous
"""Trainium2 Bass kernel for nn_BurgersSolver_75333726371954.

Burgers' equation explicit solver: interpolate u0 [64,512] to a 513-point
grid, run 5000 sequential periodic-stencil steps on [64,512], snapshot every
50th step at every 2nd spatial point -> [64,257,101].

Strategy (pure data parallel, batch sharded 8 rows/core across 8 cores):
  * Scaled state w = C1*u so the update is
        w' = LIN*w + w_l*(C2+w) + w_r*(C2-w),   LIN = 1-2*C2
  * A custom DVE op (BURGERS_STEP2_ANT) computes one full time step per
    PASS over the stream, using two depth-1 temporal feedback taps
    (CURR_ALU_OUT z^-1 of the input stream for the left term, and z^-1 of
    an intermediate accumulator for combining the right term).  One input
    stream + one output stream; out base = in base - 1.
  * MULTI-PASS instructions: all APs get a stride-0 middle dim [K=10], so
    ONE instruction = 10 time steps (the ~75ns/instruction sequencer
    overhead and the per-pass ~13ns row-restart are the only overheads
    beyond the ~1.05ns/element 1x DVE rate).  K <= 12 (hw row-count limit)
    and stream width >= ~112 elements (the src prefetcher runs ~110
    elements ahead of compute; narrower passes read stale data).
  * Layout [128 partitions = 8 batch x 16 spatial chunks of 32 sites,
    free = 2 dead cols + (H | 32 core | H) + 6 pad].  Ghost depth H=40
    allows 40 steps (4 instructions) between halo refreshes; the valid
    region tapers by 1/side/step.
  * Halo refresh via 4 DVE stream_shuffle copies (partition rotation +-1
    and +-2 within each 16-chunk group -- ghost depth 40 > chunk size 32
    needs two hops).  Everything runs on the vector engine in program
    order -- no PE, no PSUM, no cross-engine semaphores.
  * Snapshots every 50 steps land exactly on instruction boundaries
    (50 = 5 x K); strided 1x tensor-add into an SBUF accumulator, single
    DMA out at the end.  Host rescales by 1/C1 and assembles the output.
"""

import numpy as np

# ---- problem constants (hardcoded; must match the reference config) ----
MX = 513
MT = 5001
DX = 1.0 / (MX - 1)
DT = 1.0 / (MT - 1)
C1 = DT / (2.0 * DX)            # 0.0512
C2 = 0.005 * DT / DX ** 2       # 0.262144
LIN = float(1.0 - 2.0 * C2)

NSTEPS = MT - 1                 # 5000
SNAP_EVERY = 50
NSNAP = NSTEPS // SNAP_EVERY + 1  # 101

NCORES = 8
BPC = 8                         # batch rows per core
NCHUNK = 16                     # spatial chunks per batch row
CH = 32                         # chunk width (NCHUNK*CH == 512)
H = 25                          # ghost depth == steps per block
KSPLIT = (12, 13)               # passes per DVE instruction (sum == H; <= 13)
SW = CH + 2 * H                 # live state width (84)
PAD = 28                        # junk pad to keep the stream above the
                                # ~104-element src-prefetch staleness floor
WL = SW + PAD                   # instruction stream width (114)
TB = 2                          # dead leading cols (garbage landing zone)
TW = TB + WL                    # tile free width

_COMPILED = {}

# ---------------------------------------------------------------------------
# Custom DVE op: one Burgers step per pass, single input stream.
#
#   w'[c] = LIN*w[c] + w[c-1]*(C2+w[c]) + w[c+1]*(C2-w[c])
#
# Stream x = in0 (base b).  At position t (cur = x[b+t], xp = z^-1):
#   t_a = C2 + cur; t_b = C2 - xp; Q = xp*t_a; P = cur*t_b
#   A = LIN*cur + Q;  out[t] = z^-1(A) + P     -> center c = b+t-1
# out positions 0,1 are garbage (stale taps) and land on taper columns.
# ---------------------------------------------------------------------------

OP_NAME = "BURGERS_STEP2_ANT"


def _stencil_ref(in0, in1, s0, s1, imm2):
    out = np.zeros_like(in0)
    x = in0
    cur, xl, xr = x[:, 1:-1], x[:, :-2], x[:, 2:]
    out[:, 2:] = s0 * cur + xl * (s1 + cur) + xr * (s1 - cur)
    return out


def _build_uops():
    from concourse.dve_uop import (
        ENABLE, AluInp, AluOp, DelayInp, InpSel, OutPath, OutSel, Trigger,
        UopConfig,
    )

    u = UopConfig()
    u.enable_input(InpSel.SRC_0, 1)    # lane1 -> blk0's PREV_DELAY_0 port
    u.enable_input(InpSel.CONST_0, 2)  # LIN   -> PREV_DELAY_1
    u.enable_input(InpSel.CONST_1, 3)  # C2    -> PREV_DELAY_2
    u.require_inp0 = ENABLE
    u.trigger = (Trigger.SRC_TENSOR_DONE, Trigger.NONE, Trigger.NONE)
    u.next_uop = (0, 0, 0)
    u.enable_output(OutSel.ALU_OUT, OutPath.WR0_LO)
    dp = u.datapath_config
    # slice 0: stream -> flop0 (cur); d0 = z^-1 = xp; constants ride d1, d2
    dp[0].enable_alu(AluOp.BYPASS, AluInp.PREV_DELAY_0)
    dp[0].enable_delay_from_src(DelayInp.CURR_ALU_OUT, 0)    # d0 = xp
    dp[0].pass_through_delay(1, 2)
    # slice 1: t_a = cur + C2 ; keep cur in d3
    dp[1].enable_alu(AluOp.ADD, AluInp.PREV_ALU_OUT, AluInp.PREV_DELAY_2)
    dp[1].enable_delay_from_src(DelayInp.PREV_ALU_OUT, 3)    # d3 = cur
    dp[1].pass_through_delay(0, 1, 2)
    # slice 2: t_b = C2 - xp ; keep t_a in d4
    dp[2].enable_alu(AluOp.SUBTRACT, AluInp.PREV_DELAY_2, AluInp.PREV_DELAY_0)
    dp[2].enable_delay_from_src(DelayInp.PREV_ALU_OUT, 4)    # d4 = t_a
    dp[2].pass_through_delay(0, 1, 3)
    # slice 3: Q = xp * t_a ; keep t_b in d5
    dp[3].enable_alu(AluOp.MULTIPLY, AluInp.PREV_DELAY_0, AluInp.PREV_DELAY_4)
    dp[3].enable_delay_from_src(DelayInp.PREV_ALU_OUT, 5)    # d5 = t_b
    dp[3].pass_through_delay(1, 3)
    # slice 4: P = cur * t_b ; keep Q in d0
    dp[4].enable_alu(AluOp.MULTIPLY, AluInp.PREV_DELAY_3, AluInp.PREV_DELAY_5)
    dp[4].enable_delay_from_src(DelayInp.PREV_ALU_OUT, 0)    # d0 = Q
    dp[4].pass_through_delay(1, 3)
    # slice 5: A_lin = LIN * cur ; keep P in d1
    dp[5].enable_alu(AluOp.MULTIPLY, AluInp.PREV_DELAY_1, AluInp.PREV_DELAY_3)
    dp[5].enable_delay_from_src(DelayInp.PREV_ALU_OUT, 1)    # d1 = P
    dp[5].pass_through_delay(0)
    # slice 6: A = A_lin + Q ; d2 = z^-1(A) from own flop
    dp[6].enable_alu(AluOp.ADD, AluInp.PREV_ALU_OUT, AluInp.PREV_DELAY_0)
    dp[6].enable_delay_from_src(DelayInp.CURR_ALU_OUT, 2)    # d2 = A[t-1]
    dp[6].pass_through_delay(1)
    # slice 7: out = z^-1(A) + P
    dp[7].enable_alu(AluOp.ADD, AluInp.PREV_DELAY_2, AluInp.PREV_DELAY_1)
    u.validate("v3")
    return [u]


class _RawDveOp:
    """Duck-typed DveOp whose compile() returns hand-built uops."""

    def __init__(self, name):
        from concourse.dve_spec import Spec, Src0

        self.name = name
        self.subdim = False
        self.perf_en = {}
        self.spec = Spec(body=Src0, reference=_stencil_ref)
        self._compiled = {}

    def compile(self, ver):
        if ver not in self._compiled:
            from concourse.dve_ops import get_dve_sub_opcode
            from concourse.dve_uop import DveOpSpec

            self._compiled[ver] = DveOpSpec(
                name=self.name,
                opcode=get_dve_sub_opcode(self.name),
                uops=_build_uops(),
                rd1_en=False,
            )
        return self._compiled[ver]


def _register_stencil_op():
    import concourse.dve_ops as dve_ops

    for op in dve_ops.OPS:
        if op.name == OP_NAME:
            return op
    op = _RawDveOp(OP_NAME)
    dve_ops.OPS.append(op)
    dve_ops.CUSTOM_DVE_SPECS[OP_NAME] = op.spec
    dve_ops._SUB_OPCODE_FOR_NAME[OP_NAME] = (
        max(dve_ops._SUB_OPCODE_FOR_NAME.values()) + 1
    )
    assert dve_ops._SUB_OPCODE_FOR_NAME[OP_NAME] < 0x20
    return op


def _rot_mask(d):
    """out lane l takes input lane (l%16 + d)%16 within its 16-chunk group."""
    return [(l // 16) * 16 + ((l % 16) + d) % 16 for l in range(32)]


# ghost refresh pieces: (dst_lo, dst_hi, src_lo, rotation)  [state cols]
# left ghosts [0,H)  <- chunk c-1 cores [CH, CH+H) = state [2H-... wait:
#   ghost col g in [0,H): site offset g-H -> chunk c-1 col g-H+CH
#   = state col g-H+CH+H = g+CH
# right ghosts [H+CH, SW) <- chunk c+1 cols [0, H) = state cols [H, 2H)
_GHOSTS = [
    (0, H, CH, -1),               # left  [0,25)  <- c-1 state [32,57)
    (H + CH, SW, H, +1),          # right [57,84) <- c+1 state [25,50)
]


def _build():
    import concourse.bass as bass
    import concourse.mybir as mybir
    from concourse.ap import AP

    stencil = _register_stencil_op()

    F32 = mybir.dt.float32

    nc = bass.Bass()
    x_in = nc.dram_tensor("x", [128, TW], F32, kind="ExternalInput")
    y_out = nc.dram_tensor("y", [128, NSNAP * 16], F32, kind="ExternalOutput")

    n_blocks = NSTEPS // H
    assert NSTEPS % H == 0 and sum(KSPLIT) == H and SNAP_EVERY % H == 0

    with (
        nc.semaphore("dma_sem") as dma_sem,
        nc.semaphore("v_sem") as v_sem,
        nc.sbuf_tensor("U", [128, TW], F32) as U,
        nc.sbuf_tensor("SN", [128, NSNAP * 16], F32) as SN,
        nc.sbuf_tensor("ZZ", [128, 1], F32) as ZZ,
        nc.sbuf_tensor("SC", [128, 4], F32) as SC,
    ):
        ub = U[:]
        ps = ub.ap[0][0]
        aps = {k: (AP(ub.tensor, TB, [[ps, 128], [0, k], [1, WL]]),
                   AP(ub.tensor, TB - 1, [[ps, 128], [0, k], [1, WL]]))
               for k in set(KSPLIT)}

        with nc.Block() as block:
            @block.gpsimd
            def _(g):
                g.memset(ZZ[:], 0.0)
                g.memset(SC[:], 0.0)
                g.dma_start(U[:], x_in[:]).then_inc(dma_sem, 16)

            zbc = ZZ[:].to_broadcast([128, 16])
            zbc2 = ZZ[:].to_broadcast([128, 2])

            @block.vector
            def _(v):
                v.wait_ge(dma_sem, 16)

                def snapshot(k):
                    # 1x tensor_tensor add (not a 2x copy) for writeback margin
                    v.tensor_add(SN[:, k * 16:k * 16 + 16],
                                 U[:, TB + H:TB + H + CH:2], zbc)

                snapshot(0)
                snap = 1
                step = 0
                for blk in range(n_blocks):
                    for k in KSPLIT:
                        in0, out = aps[k]
                        v._custom_dve(stencil, out=out, in0=in0,
                                      s0=LIN, s1=C2)
                        step += k
                    if blk < n_blocks - 1:
                        for dlo, dhi, slo, rot in _GHOSTS:
                            w = dhi - dlo
                            v.stream_shuffle(U[:, TB + dlo:TB + dhi],
                                             U[:, TB + slo:TB + slo + w],
                                             _rot_mask(rot))
                    # snapshot doubles as the writeback-margin spacer between
                    # the ghost shuffles and the next step instruction's
                    # prefetch; on non-snapshot blocks use a dummy spacer
                    if step % SNAP_EVERY == 0:
                        snapshot(snap)
                        snap += 1
                    elif blk < n_blocks - 1:
                        v.tensor_add(SC[:, 0:2], SC[:, 2:4], zbc2)
                assert snap == NSNAP, snap
                v.tensor_add(SC[:, 0:2], SC[:, 2:4], zbc2).then_inc(v_sem, 1)

            @block.gpsimd
            def _(g):
                g.wait_ge(v_sem, 1)
                g.dma_start(y_out[:], SN[:]).then_inc(dma_sem, 16)
                g.wait_ge(dma_sem, 32)

    mybir.codegen_inst_isa_subclasses(nc)
    return nc


def _interp_init(u0):
    """Replicate the reference's 1D border-padded linear interp, f32."""
    u0 = np.asarray(u0, dtype=np.float32)
    n_in = u0.shape[1]
    X = np.linspace(0.0, 1.0, MX, dtype=np.float32)
    pts = X * np.float32(2.0) - np.float32(1.0)
    idx = (pts + np.float32(1.0)) * np.float32(0.5) * np.float32(n_in - 1)
    idx = np.clip(idx, 0.0, np.float32(n_in - 1))
    i0 = np.floor(idx).astype(np.int32)
    i0 = np.clip(i0, 0, n_in - 2)
    frac = (idx - i0.astype(np.float32)).astype(np.float32)
    u0f = u0[:, i0] * (np.float32(1.0) - frac) + u0[:, i0 + 1] * frac
    return u0f[:, :-1].astype(np.float32)   # [B, 512]


def _in_maps(u0):
    """Per-core input tiles [128, TW]: dead cols + prefilled ghosts + pad."""
    u_init = _interp_init(u0)                       # [64, 512]
    w0 = (np.float32(C1) * u_init).astype(np.float32)
    cc, xx = np.meshgrid(np.arange(NCHUNK), np.arange(TW), indexing="ij")
    src = (cc * CH + xx - TB - H) % 512             # [16, TW]
    maps = []
    for core in range(NCORES):
        wrows = w0[core * BPC:(core + 1) * BPC]     # [8, 512]
        tile = wrows[:, src].astype(np.float32)     # [8, 16, TW]
        maps.append({"x": tile.reshape(128, TW)})
    return maps


def kernel(u0):
    from concourse.bass_utils import run_bass_kernel_spmd

    u0 = np.asarray(u0, dtype=np.float32)
    B = u0.shape[0]
    assert B == NCORES * BPC and u0.shape[1] == 512

    if "nc" not in _COMPILED:
        _COMPILED["nc"] = _build()
    nc = _COMPILED["nc"]

    res = run_bass_kernel_spmd(nc, _in_maps(u0), core_ids=list(range(NCORES)))

    out = np.empty((B, 257, NSNAP), dtype=np.float32)
    inv_c1 = np.float32(1.0 / C1)
    for core in range(NCORES):
        y = res.results[core]["y"]                  # [128, NSNAP*16]
        y = y.reshape(BPC, NCHUNK, NSNAP, 16)       # [b, chunk, t, k]
        u = y * inv_c1
        # spatial index nx = chunk*16 + k  (covers 0..255)
        out[core * BPC:(core + 1) * BPC, 0:256, :] = (
            u.transpose(0, 1, 3, 2).reshape(BPC, 256, NSNAP))
    out[:, 256, :] = out[:, 0, :]
    return out


# revision 16
# speedup vs baseline: 4.7187x; 1.0058x over previous
"""Trainium2 Bass kernel for nn_BurgersSolver_75333726371954.

Burgers' equation explicit solver: interpolate u0 [64,512] to a 513-point
grid, run 5000 sequential periodic-stencil steps on [64,512], snapshot every
50th step at every 2nd spatial point -> [64,257,101].

Strategy (pure data parallel, batch sharded 8 rows/core across 8 cores):
  * Scaled state w = C1*u so the update is
        w' = LIN*w + w_l*(C2+w) + w_r*(C2-w),   LIN = 1-2*C2
  * A custom DVE op (BURGERS_STEP2_ANT) computes one full time step per
    PASS over the stream, using two depth-1 temporal feedback taps
    (CURR_ALU_OUT z^-1 of the input stream for the left term, and z^-1 of
    an intermediate accumulator for combining the right term).  One input
    stream + one output stream; out base = in base - 1.
  * MULTI-PASS instructions: all APs get a stride-0 middle dim [K=10], so
    ONE instruction = 10 time steps (the ~75ns/instruction sequencer
    overhead and the per-pass ~13ns row-restart are the only overheads
    beyond the ~1.05ns/element 1x DVE rate).  K <= 12 (hw row-count limit)
    and stream width >= ~112 elements (the src prefetcher runs ~110
    elements ahead of compute; narrower passes read stale data).
  * Layout [128 partitions = 8 batch x 16 spatial chunks of 32 sites,
    free = 2 dead cols + (H | 32 core | H) + 6 pad].  Ghost depth H=40
    allows 40 steps (4 instructions) between halo refreshes; the valid
    region tapers by 1/side/step.
  * Halo refresh via 4 DVE stream_shuffle copies (partition rotation +-1
    and +-2 within each 16-chunk group -- ghost depth 40 > chunk size 32
    needs two hops).  Everything runs on the vector engine in program
    order -- no PE, no PSUM, no cross-engine semaphores.
  * Snapshots every 50 steps land exactly on instruction boundaries
    (50 = 5 x K); strided 1x tensor-add into an SBUF accumulator, single
    DMA out at the end.  Host rescales by 1/C1 and assembles the output.
"""

import numpy as np

# ---- problem constants (hardcoded; must match the reference config) ----
MX = 513
MT = 5001
DX = 1.0 / (MX - 1)
DT = 1.0 / (MT - 1)
C1 = DT / (2.0 * DX)            # 0.0512
C2 = 0.005 * DT / DX ** 2       # 0.262144
LIN = float(1.0 - 2.0 * C2)

NSTEPS = MT - 1                 # 5000
SNAP_EVERY = 50
NSNAP = NSTEPS // SNAP_EVERY + 1  # 101

NCORES = 8
BPC = 8                         # batch rows per core
NCHUNK = 16                     # spatial chunks per batch row
CH = 32                         # chunk width (NCHUNK*CH == 512)
H = 25                          # ghost depth == steps per block
KSPLIT = (12, 13)               # passes per DVE instruction (sum == H; <= 13)
SW = CH + 2 * H                 # live state width (84)
PAD = 28                        # junk pad to keep the stream above the
                                # ~104-element src-prefetch staleness floor
WL = SW + PAD                   # instruction stream width (114)
TB = 2                          # dead leading cols (garbage landing zone)
TW = TB + WL                    # tile free width

_COMPILED = {}

# ---------------------------------------------------------------------------
# Custom DVE op: one Burgers step per pass, single input stream.
#
#   w'[c] = LIN*w[c] + w[c-1]*(C2+w[c]) + w[c+1]*(C2-w[c])
#
# Stream x = in0 (base b).  At position t (cur = x[b+t], xp = z^-1):
#   t_a = C2 + cur; t_b = C2 - xp; Q = xp*t_a; P = cur*t_b
#   A = LIN*cur + Q;  out[t] = z^-1(A) + P     -> center c = b+t-1
# out positions 0,1 are garbage (stale taps) and land on taper columns.
# ---------------------------------------------------------------------------

OP_NAME = "BURGERS_STEP2_ANT"


def _stencil_ref(in0, in1, s0, s1, imm2):
    out = np.zeros_like(in0)
    x = in0
    cur, xl, xr = x[:, 1:-1], x[:, :-2], x[:, 2:]
    out[:, 2:] = s0 * cur + xl * (s1 + cur) + xr * (s1 - cur)
    return out


def _build_uops():
    from concourse.dve_uop import (
        ENABLE, AluInp, AluOp, DelayInp, InpSel, OutPath, OutSel, Trigger,
        UopConfig,
    )

    u = UopConfig()
    u.enable_input(InpSel.SRC_0, 1)    # lane1 -> blk0's PREV_DELAY_0 port
    u.enable_input(InpSel.CONST_0, 2)  # LIN   -> PREV_DELAY_1
    u.enable_input(InpSel.CONST_1, 3)  # C2    -> PREV_DELAY_2
    u.require_inp0 = ENABLE
    u.trigger = (Trigger.SRC_TENSOR_DONE, Trigger.NONE, Trigger.NONE)
    u.next_uop = (0, 0, 0)
    u.enable_output(OutSel.ALU_OUT, OutPath.WR0_LO)
    dp = u.datapath_config
    # slice 0: stream -> flop0 (cur); d0 = z^-1 = xp; constants ride d1, d2
    dp[0].enable_alu(AluOp.BYPASS, AluInp.PREV_DELAY_0)
    dp[0].enable_delay_from_src(DelayInp.CURR_ALU_OUT, 0)    # d0 = xp
    dp[0].pass_through_delay(1, 2)
    # slice 1: t_a = cur + C2 ; keep cur in d3
    dp[1].enable_alu(AluOp.ADD, AluInp.PREV_ALU_OUT, AluInp.PREV_DELAY_2)
    dp[1].enable_delay_from_src(DelayInp.PREV_ALU_OUT, 3)    # d3 = cur
    dp[1].pass_through_delay(0, 1, 2)
    # slice 2: t_b = C2 - xp ; keep t_a in d4
    dp[2].enable_alu(AluOp.SUBTRACT, AluInp.PREV_DELAY_2, AluInp.PREV_DELAY_0)
    dp[2].enable_delay_from_src(DelayInp.PREV_ALU_OUT, 4)    # d4 = t_a
    dp[2].pass_through_delay(0, 1, 3)
    # slice 3: Q = xp * t_a ; keep t_b in d5
    dp[3].enable_alu(AluOp.MULTIPLY, AluInp.PREV_DELAY_0, AluInp.PREV_DELAY_4)
    dp[3].enable_delay_from_src(DelayInp.PREV_ALU_OUT, 5)    # d5 = t_b
    dp[3].pass_through_delay(1, 3)
    # slice 4: P = cur * t_b ; keep Q in d0
    dp[4].enable_alu(AluOp.MULTIPLY, AluInp.PREV_DELAY_3, AluInp.PREV_DELAY_5)
    dp[4].enable_delay_from_src(DelayInp.PREV_ALU_OUT, 0)    # d0 = Q
    dp[4].pass_through_delay(1, 3)
    # slice 5: A_lin = LIN * cur ; keep P in d1
    dp[5].enable_alu(AluOp.MULTIPLY, AluInp.PREV_DELAY_1, AluInp.PREV_DELAY_3)
    dp[5].enable_delay_from_src(DelayInp.PREV_ALU_OUT, 1)    # d1 = P
    dp[5].pass_through_delay(0)
    # slice 6: A = A_lin + Q ; d2 = z^-1(A) from own flop
    dp[6].enable_alu(AluOp.ADD, AluInp.PREV_ALU_OUT, AluInp.PREV_DELAY_0)
    dp[6].enable_delay_from_src(DelayInp.CURR_ALU_OUT, 2)    # d2 = A[t-1]
    dp[6].pass_through_delay(1)
    # slice 7: out = z^-1(A) + P
    dp[7].enable_alu(AluOp.ADD, AluInp.PREV_DELAY_2, AluInp.PREV_DELAY_1)
    u.validate("v3")
    return [u]


COPY_NAME = "COPY1X_ANT"


def _build_copy_uops():
    from concourse.dve_uop import (
        ENABLE, AluInp, AluOp, InpSel, OutPath, OutSel, Trigger, UopConfig,
    )

    u = UopConfig()
    u.enable_input(InpSel.SRC_0, 1)
    u.require_inp0 = ENABLE
    u.trigger = (Trigger.SRC_TENSOR_DONE, Trigger.NONE, Trigger.NONE)
    u.next_uop = (0, 0, 0)
    u.enable_output(OutSel.ALU_OUT, OutPath.WR0_LO)
    dp = u.datapath_config
    dp[0].enable_alu(AluOp.BYPASS, AluInp.PREV_DELAY_0)
    for i in range(1, 8):
        dp[i].enable_alu(AluOp.BYPASS, AluInp.PREV_ALU_OUT)
    u.validate("v3")
    return [u]


class _RawDveOp:
    """Duck-typed DveOp whose compile() returns hand-built uops."""

    def __init__(self, name, uop_builder, reference):
        from concourse.dve_spec import Spec, Src0

        self.name = name
        self.subdim = False
        self.perf_en = {}
        self.spec = Spec(body=Src0, reference=reference)
        self._uop_builder = uop_builder
        self._compiled = {}

    def compile(self, ver):
        if ver not in self._compiled:
            from concourse.dve_ops import get_dve_sub_opcode
            from concourse.dve_uop import DveOpSpec

            self._compiled[ver] = DveOpSpec(
                name=self.name,
                opcode=get_dve_sub_opcode(self.name),
                uops=self._uop_builder(),
                rd1_en=False,
            )
        return self._compiled[ver]


def _register_op(name, uop_builder, reference):
    import concourse.dve_ops as dve_ops

    for op in dve_ops.OPS:
        if op.name == name:
            return op
    op = _RawDveOp(name, uop_builder, reference)
    dve_ops.OPS.append(op)
    dve_ops.CUSTOM_DVE_SPECS[name] = op.spec
    dve_ops._SUB_OPCODE_FOR_NAME[name] = (
        max(dve_ops._SUB_OPCODE_FOR_NAME.values()) + 1
    )
    assert dve_ops._SUB_OPCODE_FOR_NAME[name] < 0x20
    return op


def _register_stencil_op():
    return _register_op(OP_NAME, _build_uops, _stencil_ref)


def _register_copy_op():
    return _register_op(COPY_NAME, _build_copy_uops,
                        lambda in0, in1, s0, s1, imm2: in0)


def _rot_mask(d):
    """out lane l takes input lane (l%16 + d)%16 within its 16-chunk group."""
    return [(l // 16) * 16 + ((l % 16) + d) % 16 for l in range(32)]


# ghost refresh pieces: (dst_lo, dst_hi, src_lo, rotation)  [state cols]
# left ghosts [0,H)  <- chunk c-1 cores [CH, CH+H) = state [2H-... wait:
#   ghost col g in [0,H): site offset g-H -> chunk c-1 col g-H+CH
#   = state col g-H+CH+H = g+CH
# right ghosts [H+CH, SW) <- chunk c+1 cols [0, H) = state cols [H, 2H)
_GHOSTS = [
    (0, H, CH, -1),               # left  [0,25)  <- c-1 state [32,57)
    (H + CH, SW, H, +1),          # right [57,84) <- c+1 state [25,50)
]


def _build():
    import concourse.bass as bass
    import concourse.mybir as mybir
    from concourse.ap import AP

    stencil = _register_stencil_op()
    copy1x = _register_copy_op()

    F32 = mybir.dt.float32

    nc = bass.Bass()
    x_in = nc.dram_tensor("x", [128, TW], F32, kind="ExternalInput")
    y_out = nc.dram_tensor("y", [128, NSNAP * 16], F32, kind="ExternalOutput")

    n_blocks = NSTEPS // H
    assert NSTEPS % H == 0 and sum(KSPLIT) == H and SNAP_EVERY % H == 0

    with (
        nc.semaphore("dma_sem") as dma_sem,
        nc.semaphore("v_sem") as v_sem,
        nc.sbuf_tensor("U", [128, TW], F32) as U,
        nc.sbuf_tensor("SN", [128, NSNAP * 16], F32) as SN,
        nc.sbuf_tensor("ZZ", [128, 1], F32) as ZZ,
        nc.sbuf_tensor("SC", [128, 4], F32) as SC,
    ):
        ub = U[:]
        ps = ub.ap[0][0]
        aps = {k: (AP(ub.tensor, TB, [[ps, 128], [0, k], [1, WL]]),
                   AP(ub.tensor, TB - 1, [[ps, 128], [0, k], [1, WL]]))
               for k in set(KSPLIT)}

        with nc.Block() as block:
            @block.gpsimd
            def _(g):
                g.memset(ZZ[:], 0.0)
                g.memset(SC[:], 0.0)
                g.dma_start(U[:], x_in[:]).then_inc(dma_sem, 16)

            zbc = ZZ[:].to_broadcast([128, 16])
            zbc2 = ZZ[:].to_broadcast([128, 2])

            @block.vector
            def _(v):
                v.wait_ge(dma_sem, 16)

                def snapshot(k):
                    # custom 1x copy (a stock 2x copy could outrun writeback)
                    v._custom_dve(copy1x, out=SN[:, k * 16:k * 16 + 16],
                                  in0=U[:, TB + H:TB + H + CH:2])

                snapshot(0)
                snap = 1
                step = 0
                for blk in range(n_blocks):
                    for k in KSPLIT:
                        in0, out = aps[k]
                        v._custom_dve(stencil, out=out, in0=in0,
                                      s0=LIN, s1=C2)
                        step += k
                    if blk < n_blocks - 1:
                        for dlo, dhi, slo, rot in _GHOSTS:
                            w = dhi - dlo
                            v.stream_shuffle(U[:, TB + dlo:TB + dhi],
                                             U[:, TB + slo:TB + slo + w],
                                             _rot_mask(rot))
                    # snapshot doubles as the writeback-margin spacer between
                    # the ghost shuffles and the next step instruction's
                    # prefetch; on non-snapshot blocks use a dummy spacer
                    if step % SNAP_EVERY == 0:
                        snapshot(snap)
                        snap += 1
                    elif blk < n_blocks - 1:
                        v._custom_dve(copy1x, out=SC[:, 0:2], in0=SC[:, 2:4])
                assert snap == NSNAP, snap
                v.tensor_add(SC[:, 0:2], SC[:, 2:4], zbc2).then_inc(v_sem, 1)

            @block.gpsimd
            def _(g):
                g.wait_ge(v_sem, 1)
                g.dma_start(y_out[:], SN[:]).then_inc(dma_sem, 16)
                g.wait_ge(dma_sem, 32)

    mybir.codegen_inst_isa_subclasses(nc)
    return nc


def _interp_init(u0):
    """Replicate the reference's 1D border-padded linear interp, f32."""
    u0 = np.asarray(u0, dtype=np.float32)
    n_in = u0.shape[1]
    X = np.linspace(0.0, 1.0, MX, dtype=np.float32)
    pts = X * np.float32(2.0) - np.float32(1.0)
    idx = (pts + np.float32(1.0)) * np.float32(0.5) * np.float32(n_in - 1)
    idx = np.clip(idx, 0.0, np.float32(n_in - 1))
    i0 = np.floor(idx).astype(np.int32)
    i0 = np.clip(i0, 0, n_in - 2)
    frac = (idx - i0.astype(np.float32)).astype(np.float32)
    u0f = u0[:, i0] * (np.float32(1.0) - frac) + u0[:, i0 + 1] * frac
    return u0f[:, :-1].astype(np.float32)   # [B, 512]


def _in_maps(u0):
    """Per-core input tiles [128, TW]: dead cols + prefilled ghosts + pad."""
    u_init = _interp_init(u0)                       # [64, 512]
    w0 = (np.float32(C1) * u_init).astype(np.float32)
    cc, xx = np.meshgrid(np.arange(NCHUNK), np.arange(TW), indexing="ij")
    src = (cc * CH + xx - TB - H) % 512             # [16, TW]
    maps = []
    for core in range(NCORES):
        wrows = w0[core * BPC:(core + 1) * BPC]     # [8, 512]
        tile = wrows[:, src].astype(np.float32)     # [8, 16, TW]
        maps.append({"x": tile.reshape(128, TW)})
    return maps


def kernel(u0):
    from concourse.bass_utils import run_bass_kernel_spmd

    u0 = np.asarray(u0, dtype=np.float32)
    B = u0.shape[0]
    assert B == NCORES * BPC and u0.shape[1] == 512

    if "nc" not in _COMPILED:
        _COMPILED["nc"] = _build()
    nc = _COMPILED["nc"]

    res = run_bass_kernel_spmd(nc, _in_maps(u0), core_ids=list(range(NCORES)))

    out = np.empty((B, 257, NSNAP), dtype=np.float32)
    inv_c1 = np.float32(1.0 / C1)
    for core in range(NCORES):
        y = res.results[core]["y"]                  # [128, NSNAP*16]
        y = y.reshape(BPC, NCHUNK, NSNAP, 16)       # [b, chunk, t, k]
        u = y * inv_c1
        # spatial index nx = chunk*16 + k  (covers 0..255)
        out[core * BPC:(core + 1) * BPC, 0:256, :] = (
            u.transpose(0, 1, 3, 2).reshape(BPC, 256, NSNAP))
    out[:, 256, :] = out[:, 0, :]
    return out
